# revision 53
# baseline (speedup 1.0000x reference)
"""Trainium2 Bass kernel for a 2-layer CIN (Compressed Interaction Network).

Reference computation (per batch b, embedding dim d):
    h1[q] = sum_{f,g} x[f] x[g] W0[q, f*39+g]          (f,g in 0..38)
    h2[h] = sum_{f,q} x[f] h1[q] W1[h, f*128+q]        (f in 0..38, q in 0..127)
    out[b] = concat(sum_d h1, sum_d h2)                 -> [B, 256]

Device mapping (data-parallel over batch across 8 cores, 256 b's each):
  * Layer 1 uses a polarization ("sum of squares") identity so the outer
    product x (x) x never materializes:  x_i x_j = ((x_i+x_j)^2 - x_i^2 - x_j^2)/2.
    With 780 fixed linear forms V (39 singles + 741 pair sums, padded to
    896 = 7*128) and re-packed coefficients C:  h1 = C^T (V^T x)^2.
    All matmuls padded to contraction k=128 (k<128 measured 3.7x slower).
  * Layer 2 exploits  sum_d h2[b,:,d] = W1flat @ vec(S_b),
    S_b[f,q] = sum_d x[b,f,d] h1[b,q,d]  (a k=16 outer-product contraction
    per batch).  S^T is computed 3 batches at a time with one k=128 matmul
    against a host-precomputed block-diagonal transposed-x operand, after
    transposing h1 on the PE.  The final contraction is 39 k=128 matmuls.
"""

import numpy as np

import concourse.mybir as mybir
import concourse.tile as tile
from concourse import bacc
from concourse.bass import ts
from concourse.bass_utils import run_bass_kernel_spmd

B, F0, D = 2048, 39, 16
H1, H2 = 128, 128
NCORES = 8
BC = B // NCORES          # 256 batches per core
BT = 32                   # batches per tile
NT = BC // BT             # 8 tiles per core
N = BT * D                # 512 columns per tile (cols = (b, d), d inner)
NFP = 896                 # forms padded to 7*128
NCHUNK = 7
CW = 128                  # forms per chunk
GB = 4                    # batches per S-chunk (4*32 = 128 partitions exactly)
NG = 8                    # S-chunks per tile (8*4 = 32)
DP = 32                   # padded d-block (16 real + 16 zero)

F16 = mybir.dt.float16
F32 = mybir.dt.float32


def pack_weights(W0: np.ndarray, W1: np.ndarray):
    """Host-side repack of CIN weights into device layouts (fp16)."""
    W0m = W0[:, :, 0].reshape(H1, F0, F0).astype(np.float64)
    W1m = W1[:, :, 0].reshape(H2, F0, H1).astype(np.float64)

    V = np.zeros((128, NFP), dtype=np.float64)   # k-padded: rows 39.. = 0
    C = np.zeros((NFP, H1), dtype=np.float64)
    for i in range(F0):
        V[i, i] = 1.0
        Bi = W0m[:, i, :] + W0m[:, :, i]          # [H, F]
        C[i, :] = W0m[:, i, i] - 0.5 * (Bi.sum(axis=1) - 2.0 * W0m[:, i, i])
    k = F0
    for i in range(F0):
        for j in range(i + 1, F0):
            V[i, k] = 1.0
            V[j, k] = 1.0
            C[k, :] = 0.5 * (W0m[:, i, j] + W0m[:, j, i])
            k += 1
    c_pack = C.reshape(NCHUNK, CW, H1).transpose(1, 0, 2)   # [128, 7, 128]

    w1p = W1m.transpose(2, 1, 0)                   # [q=128, f=39, h=128]

    ident = np.eye(128, dtype=np.float16)

    return {
        "vp": np.ascontiguousarray(V, dtype=np.float16),
        "cp": np.ascontiguousarray(c_pack, dtype=np.float16),
        "w1p": np.ascontiguousarray(w1p, dtype=np.float16),
        "ident": ident,
    }


def pack_x(x_core: np.ndarray):
    """Per-core input repack: f-padded dense x + block-diagonal transposed x.

    x_core: [BC, 39, 16] float.
    Returns xp [BC, 128, 16] fp16 (f rows 39.. zero) and
    xt3 [NT, NG, 128, 117] fp16: chunk (t,c) covers batches 32t+3c+j,
    partition p=(j*32+d), col=(j*39+f), value x[b, f, d] (zero-padded).
    """
    x16 = x_core.astype(np.float16)
    xp = np.zeros((BC, 128, D), dtype=np.float16)
    xp[:, :F0, :] = x16
    # columns j-major: col = j*F0 + f so S^T lands [q, (j, f)]
    xt3 = np.zeros((NT, NG, GB, DP, GB, F0), dtype=np.float16)
    x5 = x16.reshape(NT, BT, F0, D)
    for j in range(GB):
        bs = np.arange(NG) * GB + j
        # [NT, NG, D, F0] into block (partition j, col j)
        xt3[:, :, j, :D, j, :] = x5[:, bs].transpose(0, 1, 3, 2)
    return xp, np.ascontiguousarray(
        xt3.reshape(NT, NG, 128, GB * F0))


def build(reps: int = 1, stage: str = "full"):
    """Build the per-core Bass module. reps>1 wraps the body in a HW loop
    (wall-clock timing only — the graded path uses reps=1).
    stage: cumulative subset for profiling:
      'l1' | 'h1p3' | 'trans' | 'smm' | 'nomemset' | 'full'"""
    S = ["l1", "h1p3", "trans", "smm", "full"].index(
        "full" if stage == "nomemset" else stage)
    nc = bacc.Bacc("TRN2", target_bir_lowering=False, debug=False,
                   num_devices=NCORES)

    x_h = nc.dram_tensor("xp", [BC, 128, D], F16, kind="ExternalInput")
    xt3_h = nc.dram_tensor("xt3", [NT, NG, 128, GB * F0], F16,
                           kind="ExternalInput")
    vp_h = nc.dram_tensor("vp", [128, NFP], F16, kind="ExternalInput")
    cp_h = nc.dram_tensor("cp", [CW, NCHUNK, H1], F16, kind="ExternalInput")
    w1p_h = nc.dram_tensor("w1p", [H1, F0, H2], F16, kind="ExternalInput")
    id_h = nc.dram_tensor("ident", [128, 128], F16, kind="ExternalInput")
    out_h = nc.dram_tensor("out", [2, 128, BC], F32, kind="ExternalOutput")

    x_ap = x_h.ap().rearrange("b f d -> f b d")      # [128, 256, 16]
    xt3_ap = xt3_h.ap().rearrange("t c p w -> p t c w")  # [128, 8, 11, 117]

    with tile.TileContext(nc) as tc:
        with (
            tc.tile_pool(name="const", bufs=1) as const,
            tc.tile_pool(name="xpool", bufs=3) as xpool,
            tc.tile_pool(name="xtp", bufs=3) as xtp,
            tc.tile_pool(name="ysq", bufs=3) as ysqp,
            tc.tile_pool(name="h1p3", bufs=3) as h1p3p,
            tc.tile_pool(name="h1t", bufs=3) as h1tp,
            tc.tile_pool(name="yps", bufs=2, space="PSUM") as yps,
            tc.tile_pool(name="h1ps", bufs=2, space="PSUM") as h1psp,
            tc.tile_pool(name="h1tps", bufs=2, space="PSUM") as h1tps,
            tc.tile_pool(name="stps", bufs=2, space="PSUM") as stps,
        ):
            v_sb = const.tile([128, NFP], F16)
            nc.sync.dma_start(out=v_sb[:], in_=vp_h.ap())
            c_sb = const.tile([CW, NCHUNK, H1], F16)
            nc.sync.dma_start(out=c_sb[:], in_=cp_h.ap())
            w1_sb = const.tile([H1, F0, H2], F16)
            nc.sync.dma_start(out=w1_sb[:], in_=w1p_h.ap())
            id_sb = const.tile([128, 128], F16)
            nc.sync.dma_start(out=id_sb[:], in_=id_h.ap())
            out1_sb = const.tile([128, BC], F32)
            out2_sb = const.tile([128, BC], F32)
            sall_sb = const.tile([128, BC, F0], F16)   # S^T: [q, b, f]

            def final_half(hf):
                # out2 for b in [hf*128, hf*128+128): overlaps later tiles
                out2_ps = h1psp.tile([128, 128], F32, tag="h1_ps")
                for f in range(F0):
                    nc.tensor.matmul(out2_ps[:], w1_sb[:, f, :],
                                     sall_sb[:, ts(hf, 128), f],
                                     start=(f == 0), stop=(f == F0 - 1))
                nc.scalar.copy(out2_sb[:, ts(hf, 128)], out2_ps[:])

            def body(_i=None):
                for t in range(NT):
                    x_sb = xpool.tile([128, BT, D], F16)
                    nc.sync.dma_start(out=x_sb[:], in_=x_ap[:, ts(t, BT), :])
                    x_flat = x_sb[:, :, :]           # [128, 512]
                    xt_sb = xtp.tile([128, NG, GB * F0], F16)
                    nc.sync.dma_start(out=xt_sb[:], in_=xt3_ap[:, t])

                    # ---- layer 1: h1 = C^T (V^T x)^2, all k=128 ----
                    ysq = ysqp.tile([128, NCHUNK, N], F16)
                    for j in range(NCHUNK):
                        y_ps = yps.tile([128, N], F32, tag="y")
                        nc.tensor.matmul(y_ps[:], v_sb[:, ts(j, CW)], x_flat,
                                         start=True, stop=True)
                        nc.scalar.square(ysq[:, j, :], y_ps[:])
                    h1_ps = h1psp.tile([H1, N], F32)
                    for j in range(NCHUNK):
                        nc.tensor.matmul(h1_ps[:], c_sb[:, j, :], ysq[:, j, :],
                                         start=(j == 0), stop=(j == NCHUNK - 1))
                    nc.vector.reduce_sum(
                        out=out1_sb[:, ts(t, BT)],
                        in_=h1_ps.rearrange("p (b d) -> p b d", d=D),
                        axis=mybir.AxisListType.X,
                    )

                    # ---- h1 -> padded (3b x 32d + 32z) column blocks ----
                    if S < 1:
                        continue
                    h1p3 = h1p3p.tile([128, NG, 128], F16)
                    if stage != "nomemset":
                        nc.gpsimd.memset(h1p3[:], 0.0)
                    # cols per chunk: 4 batches x 32 padded d
                    nc.scalar.copy(
                        h1p3[:].rearrange(
                            "p c (j dp) -> p c j dp", dp=DP)[:, :, :, :D],
                        h1_ps[:].rearrange("p (c j d) -> p c j d",
                                           j=GB, d=D),
                    )

                    # ---- transpose h1 blocks; S^T-matmuls (k=128) ----
                    if S < 2:
                        continue
                    h1t_sb = h1tp.tile([128, NG, 128], F16)
                    for half in range(2):            # chunks 0-3, 4-7
                        c0 = 4 * half
                        h1t_ps = h1tps.tile([128, 4, 128], F16, tag="h1t")
                        for u in range(4):
                            nc.tensor.transpose(h1t_ps[:, u, :],
                                                h1p3[:, c0 + u, :], id_sb[:])
                        nc.vector.tensor_copy(
                            out=h1t_sb[:, c0:c0 + 4, :],
                            in_=h1t_ps[:])

                    if S < 3:
                        continue
                    for grp in range(3):             # S chunks 0-2, 3-5, 6-7
                        g0 = 3 * grp
                        ncnk = 3 if grp < 2 else 2
                        st_ps = stps.tile([128, 3, GB * F0], F32, tag="st")
                        for u in range(ncnk):
                            c = g0 + u
                            nc.tensor.matmul(st_ps[:, u, :], h1t_sb[:, c, :],
                                             xt_sb[:, c, :],
                                             start=True, stop=True)
                        # contiguous: [q, (u, j, f)] -> sall[q, b0.., f]
                        b0 = 32 * t + GB * g0
                        nc.vector.tensor_copy(
                            out=sall_sb[:, b0:b0 + GB * ncnk, :],
                            in_=st_ps[:, :ncnk, :].rearrange(
                                "p u w -> p (u w)"),
                        )

                    if S >= 4 and t == NT // 2 - 1:
                        final_half(0)   # overlap first half with tiles 4..7

                # ---- final: out2 = sum_f W1[:,f,:].T @ S^T[:, :, f] ----
                if S >= 4:
                    final_half(1)
                    nc.sync.dma_start(out=out_h.ap()[1], in_=out2_sb[:])

                nc.sync.dma_start(out=out_h.ap()[0], in_=out1_sb[:])

            if reps == 1:
                body()
            else:
                with tc.For_i(0, reps) as i:
                    body(i)

    nc.compile()
    return nc


_CACHE: dict = {}


def _get_module(reps: int = 1):
    if reps not in _CACHE:
        _CACHE[reps] = build(reps)
    return _CACHE[reps]


def run(input: np.ndarray, W0: np.ndarray, W1: np.ndarray, reps: int = 1):
    nc = _get_module(reps)
    packs = pack_weights(np.asarray(W0), np.asarray(W1))
    x_np = np.asarray(input)
    in_maps = []
    for c in range(NCORES):
        xp, xt3 = pack_x(x_np[c * BC:(c + 1) * BC])
        m = {"xp": xp, "xt3": xt3}
        m.update(packs)
        in_maps.append(m)
    res = run_bass_kernel_spmd(nc, in_maps, core_ids=list(range(NCORES)))
    out = np.empty((B, 256), dtype=np.float32)
    for c in range(NCORES):
        o = res.results[c]["out"]          # [2, 128, 256]
        out[c * BC:(c + 1) * BC, :128] = o[0].T
        out[c * BC:(c + 1) * BC, 128:] = o[1].T
    return out


def kernel(input: np.ndarray, W0: np.ndarray, W1: np.ndarray) -> np.ndarray:
    return run(input, W0, W1, reps=1)
